# revision 5
# baseline (speedup 1.0000x reference)
"""DSAttention Trainium2 kernel (v3).

Reference computation (per batch b, head h):
    S[q,s]  = (Q[q]·K[s]) * tau[b] + delta[b,s]
    S      += causal mask (s > q -> -inf)
    A       = softmax(S / sqrt(E), axis=s)
    O[q,:]  = sum_s A[q,s] * V[s,:]

Shapes: B=2, L=2048, H=16, E=64 -> 32 (b,h) pairs, 4 per NeuronCore x 8 cores.

v3 design (v2 was ACT-rebalanced but still PE-streaming-bound at ~55us):
  - The whole kernel runs the PE in 64x128 row-tiled mode (tiles T0/T8).
    qt/kt hold Q^T*(tau*log2e/8) and K^T twice (rows 0-63 and a copy at
    64-127), so the two row tiles can stream two different q-pieces of the
    same s-chunk concurrently -> QK at 2 cols/cycle. The two pieces of a
    chunk go to different PSUM banks (row-tile bank rule).
  - delta no longer occupies a contract row: the ACT path applies it as a
    per-partition activation bias (delta/8, since exp(ln2*t + delta/8)),
    the DVE path folds it into the Schraudolph add-constant AP.
  - AV runs as T0/T8 row halves (s 0-63 / 64-127 of each chunk)
    accumulating into two separate PSUM tiles oA/oB (bank rule); the
    drain merges them on DVE (add instead of copy, same cost).
  - exp split ACT/DVE as in v2 but ~37% on DVE (Schraudolph fp16
    bit-trick: int16(t*2^10 + (delta*log2e/8*2^10 + B10)) bitcast fp16).
  - Diagonal causal masks on GpSimd; output is O^T-unnormalized
    [65, L] fp32 per head (row 64 = denominator), host divides+transposes.
"""

import sys

sys.path.insert(0, "/opt/trn_rl_repo")

import numpy as np

import concourse.bass as bass
import concourse.tile as tile
from concourse import bacc, mybir
from concourse.masks import make_upper_triangular

B, L, H, E = 2, 2048, 16, 64
NCORES = 8
HPC = (B * H) // NCORES  # heads per core = 4
NCH = L // 128  # s-chunks per head = 16
LOG2E = 1.4426950408889634
LN2 = 0.6931471805599453
FOLD = LOG2E / 8.0  # folds the 1/sqrt(E) softmax scale + base-2 conversion
F32 = mybir.dt.float32
F16 = mybir.dt.float16
I16 = mybir.dt.int16
EXP = mybir.ActivationFunctionType.Exp
MULT = mybir.AluOpType.mult
ADD = mybir.AluOpType.add

# Schraudolph constant for fp16 (exp bias 15, 10 mantissa bits):
# bitcast_f16(int16(t*2^10 + B10)) ~= 2^t, C tuned for mean relative error.
B10 = 15.0 * 1024.0 - 0.00725 * 1024.0

# Chunks whose exp runs on DVE instead of ACT: (phase, n).
# 896+512 + 4*1024 + 896 = 6496/17408 = 37% of exp columns per head.
DVE_CHUNKS = {(0, 1), (0, 4), (1, 0), (1, 2), (1, 4), (1, 6), (1, 9)}


def _body(tc, qT, kT, v1, dl8, dl10, out):
    nc = tc.nc
    from contextlib import ExitStack

    with ExitStack() as ctx:
        const = ctx.enter_context(tc.tile_pool(name="const", bufs=1))
        qk_pool = ctx.enter_context(tc.tile_pool(name="qk", bufs=2))
        v_pool = ctx.enter_context(tc.tile_pool(name="v", bufs=2))
        d_pool = ctx.enter_context(tc.tile_pool(name="d", bufs=2))
        a_pool = ctx.enter_context(tc.tile_pool(name="a", bufs=3))
        o_pool = ctx.enter_context(tc.tile_pool(name="o", bufs=2))
        ps_pool = ctx.enter_context(tc.tile_pool(name="psS", bufs=2, space="PSUM"))
        po_pool = ctx.enter_context(tc.tile_pool(name="psO", bufs=1, space="PSUM"))

        trimask = const.tile([128, 128], F16, name="trimask")
        make_upper_triangular(nc, trimask[:], val=1.0, diag=True)

        for i in range(HPC):
            qt = qk_pool.tile([128, L], F16, tag="qt", name=f"qt{i}")
            kt = qk_pool.tile([128, L], F16, tag="kt", name=f"kt{i}")
            vt = v_pool.tile([128, NCH * 65], F16, tag="vt", name=f"vt{i}")
            d8 = d_pool.tile([128, NCH], F32, tag="d8", name=f"d8_{i}")
            d10 = d_pool.tile([128, NCH], F32, tag="d10", name=f"d10_{i}")
            nc.sync.dma_start(d8[:], dl8[i])
            nc.sync.dma_start(d10[:], dl10[i])
            for hf in range(2):
                cs = slice(1024 * hf, 1024 * hf + 1024)
                # duplicated 64-row halves so T8 streams from partitions 64-127
                nc.sync.dma_start(kt[0:64, cs], kT[i][:, cs])
                nc.sync.dma_start(kt[64:128, cs], kT[i][:, cs])
                nc.sync.dma_start(qt[0:64, cs], qT[i][:, cs])
                nc.sync.dma_start(qt[64:128, cs], qT[i][:, cs])
                vs = slice(8 * 65 * hf, 8 * 65 * hf + 8 * 65)
                nc.sync.dma_start(vt[:, vs], v1[i][:, vs])

            for phase in range(2):
                qlo = 1024 * phase
                qhi = qlo + 1024
                oA = po_pool.tile([65, 1024], F32, tag="oA", name=f"oA{i}_{phase}")
                oB = po_pool.tile([65, 1024], F32, tag="oB", name=f"oB{i}_{phase}")
                pend = []  # emitted QK/exp awaiting AV emission

                def emit_av(u):
                    n, pieces, a_ap = u
                    for c0, w in pieces:
                        j = (c0 - qlo) // 512
                        stop = n == 8 * phase + 4 * j + 3
                        sl = slice(c0 - qlo, c0 - qlo + w)
                        nc.tensor.matmul(
                            oA[:, sl],
                            lhsT=vt[0:64, n * 65 : n * 65 + 65],
                            rhs=a_ap[0:64, sl],
                            start=(n == 0),
                            stop=stop,
                        )
                        nc.tensor.matmul(
                            oB[:, sl],
                            lhsT=vt[64:128, n * 65 : n * 65 + 65],
                            rhs=a_ap[64:128, sl],
                            start=(n == 0),
                            stop=stop,
                        )

                for n in range(qhi // 128):
                    q0 = max(128 * n, qlo)
                    ps = ps_pool.tile(
                        [128, 1024], F32, tag="ps", name=f"ps{i}_{phase}_{n}"
                    )
                    # QK: two row tiles stream two bank-disjoint q-pieces
                    # of this s-chunk concurrently.
                    kt_lo = kt[0:64, 128 * n : 128 * n + 128]
                    kt_hi = kt[64:128, 128 * n : 128 * n + 128]
                    if q0 < qlo + 512:
                        nc.tensor.matmul(
                            ps[:, q0 - qlo : 512],
                            lhsT=kt_lo,
                            rhs=qt[0:64, q0 : qlo + 512],
                            start=True,
                            stop=True,
                        )
                        nc.tensor.matmul(
                            ps[:, 512:1024],
                            lhsT=kt_hi,
                            rhs=qt[64:128, qlo + 512 : qhi],
                            start=True,
                            stop=True,
                        )
                        pieces = [(q0, qlo + 512 - q0), (qlo + 512, 512)]
                    else:
                        nc.tensor.matmul(
                            ps[:, q0 - qlo : 1024],
                            lhsT=kt_lo,
                            rhs=qt[0:64, q0:qhi],
                            start=True,
                            stop=True,
                        )
                        pieces = [(q0, qhi - q0)]
                    sl = slice(q0 - qlo, 1024)
                    if (phase, n) in DVE_CHUNKS:
                        ai = a_pool.tile(
                            [128, 1024], I16, tag="a", name=f"a{i}_{phase}_{n}"
                        )
                        nc.vector.tensor_scalar(
                            ai[:, sl], ps[:, sl], 1024.0, d10[:, n : n + 1], MULT, ADD
                        )
                        a_ap = ai[:].bitcast(F16)
                    else:
                        af = a_pool.tile(
                            [128, 1024], F16, tag="a", name=f"a{i}_{phase}_{n}"
                        )
                        nc.scalar.activation(
                            af[:, sl], ps[:, sl], EXP,
                            bias=d8[:, n : n + 1], scale=LN2,
                        )
                        a_ap = af[:]
                    if 128 * n >= qlo:
                        nc.gpsimd.tensor_mul(
                            a_ap[:, q0 - qlo : q0 - qlo + 128],
                            a_ap[:, q0 - qlo : q0 - qlo + 128],
                            trimask[:],
                        )
                    pend.append((n, pieces, a_ap))
                    if len(pend) > 2:
                        emit_av(pend.pop(0))
                for u in pend:
                    emit_av(u)

                # TensorTensor may read only one PSUM operand: ACT copies oA
                # to SBUF, DVE accumulates oB on top.
                o_sb = o_pool.tile([65, 1024], F32, tag="osb", name=f"osb{i}_{phase}")
                nc.scalar.copy(o_sb[:], oA[:])
                nc.vector.tensor_add(o_sb[:], o_sb[:], oB[:])
                nc.sync.dma_start(out[i][:, qlo:qhi], o_sb[:])


_CACHED = None


def _build():
    global _CACHED
    if _CACHED is not None:
        return _CACHED
    nc = bacc.Bacc("TRN2", target_bir_lowering=False, debug=False)
    qT = nc.dram_tensor("qT", [HPC, 64, L], F16, kind="ExternalInput").ap()
    kT = nc.dram_tensor("kT", [HPC, 64, L], F16, kind="ExternalInput").ap()
    v1 = nc.dram_tensor("v1", [HPC, 128, NCH * 65], F16, kind="ExternalInput").ap()
    dl8 = nc.dram_tensor("dl8", [HPC, 128, NCH], F32, kind="ExternalInput").ap()
    dl10 = nc.dram_tensor("dl10", [HPC, 128, NCH], F32, kind="ExternalInput").ap()
    out = nc.dram_tensor("out", [HPC, 65, L], F32, kind="ExternalOutput").ap()
    with tile.TileContext(nc) as tc:
        _body(tc, qT, kT, v1, dl8, dl10, out)
    nc.compile()
    _CACHED = nc
    return nc


def _prep_in_maps(queries, keys, values, tau, delta):
    """Shard + relayout the full inputs into 8 per-core input dicts."""
    queries = np.asarray(queries, dtype=np.float32)
    keys = np.asarray(keys, dtype=np.float32)
    values = np.asarray(values, dtype=np.float32)
    tau = np.asarray(tau, dtype=np.float32)
    delta = np.asarray(delta, dtype=np.float32)

    in_maps = []
    for core in range(NCORES):
        qTs = np.empty((HPC, 64, L), np.float16)
        kTs = np.empty((HPC, 64, L), np.float16)
        v1s = np.empty((HPC, 128, NCH * 65), np.float16)
        d8s = np.empty((HPC, 128, NCH), np.float32)
        d10s = np.empty((HPC, 128, NCH), np.float32)
        for slot in range(HPC):
            g = core * HPC + slot
            b, h = divmod(g, H)
            qTs[slot] = queries[b, :, h, :].T * (tau[b, 0] * FOLD)
            kTs[slot] = keys[b, :, h, :].T
            v = values[b, :, h, :].reshape(NCH, 128, E).transpose(1, 0, 2)
            vv = np.concatenate([v, np.ones((128, NCH, 1), np.float32)], axis=2)
            v1s[slot] = vv.reshape(128, NCH * 65).astype(np.float16)
            dchunks = delta[b].reshape(NCH, 128).T  # [s, n]
            d8s[slot] = dchunks / 8.0
            d10s[slot] = dchunks * (FOLD * 1024.0) + B10
        in_maps.append(
            {"qT": qTs, "kT": kTs, "v1": v1s, "dl8": d8s, "dl10": d10s}
        )
    return in_maps


def _assemble(results):
    O = np.empty((B, L, H, E), np.float32)
    for core in range(NCORES):
        o = results[core]["out"]  # [HPC, 65, L]
        for slot in range(HPC):
            g = core * HPC + slot
            b, h = divmod(g, H)
            O[b, :, h, :] = (o[slot, 0:64, :] / o[slot, 64:65, :]).T
    return O


def run(inputs, trace=False, **kwargs):
    from concourse import bass_utils

    nc = _build()
    in_maps = _prep_in_maps(**inputs)
    res = bass_utils.run_bass_kernel_spmd(
        nc, in_maps, core_ids=list(range(NCORES)), trace=trace, **kwargs
    )
    return _assemble(res.results), res


def kernel(**inputs):
    return run(inputs, trace=False)[0]


# revision 8
# speedup vs baseline: 1.1010x; 1.1010x over previous
"""DSAttention Trainium2 kernel (v4).

Reference computation (per batch b, head h):
    S[q,s]  = (Q[q]·K[s]) * tau[b] + delta[b,s]
    S      += causal mask (s > q -> -inf)
    A       = softmax(S / sqrt(E), axis=s)
    O[q,:]  = sum_s A[q,s] * V[s,:]

Shapes: B=2, L=2048, H=16, E=64 -> 32 (b,h) pairs, 4 per NeuronCore x 8 cores.

v4 design. Engine-work floors per core (measured v2): PE ~55us of matmul
column streaming (QK 17408 + AV 17408 cols/head at 1 col/cycle; tiling
cannot beat this - the ifmap XBUS serializes any second stream), ACT+DVE
~58us of exp split between them, GpSimd ~30us of causal masks. v2 hit 108us
because the pipeline only kept the busiest engine 75% fed: the 2-deep
[128,1024] PSUM ring coupled QK(n+2) to the full 1024-wide exp(n), and
every phase/head boundary flushed the AV lag queue. v4 restructures the
schedule around the same math:
  - S PSUM ring: 4 x [128,512] tiles (4 banks). Each QK piece (<=512 cols)
    gets its own tile; exp runs per piece, so the producer-consumer loop
    QK(p) -> exp(p) -> QK(p+4) spans 4 pieces and stops binding the rate.
  - exp pieces alternate ACT (native Exp, scale=ln2) / DVE (Schraudolph
    fp16 bit trick written through an int16-bitcast view of the fp16 a_sb
    tile), greedily targeting ~1/3 of columns on DVE.
  - One flat emission pipeline across all (head, phase, chunk): the AV lag
    queue (4 pieces) drains into the next phase's QK stream instead of
    flushing at boundaries; oT [65,1024] double-buffered per phase.
  - All 4 heads' qt/kt/vt DMA upfront (SBUF is big enough) - no head
    boundary load stalls.
  - Layout as v2: qt/kt [65, L] fp16 (rows 0-63 Q^T*(tau*log2e/8) / K^T,
    row 64 ones / delta*log2e/8), vt [128, 16*65] with a ones column, so
    the QK matmul emits t = log2(e^(score/8)) and AV row 64 the softmax
    denominator. Host divides + transposes.
"""

import sys

sys.path.insert(0, "/opt/trn_rl_repo")

import numpy as np

import concourse.bass as bass
import concourse.tile as tile
from concourse import bacc, mybir
from concourse.masks import make_upper_triangular

B, L, H, E = 2, 2048, 16, 64
NCORES = 8
HPC = (B * H) // NCORES  # heads per core = 4
NCH = L // 128  # s-chunks per head = 16
LOG2E = 1.4426950408889634
LN2 = 0.6931471805599453
FOLD = LOG2E / 8.0  # folds the 1/sqrt(E) softmax scale + base-2 conversion
F32 = mybir.dt.float32
F16 = mybir.dt.float16
I16 = mybir.dt.int16
EXP = mybir.ActivationFunctionType.Exp
MULT = mybir.AluOpType.mult
ADD = mybir.AluOpType.add

# Schraudolph constant for fp16 (exp bias 15, 10 mantissa bits):
# bitcast_f16(int16(t*2^10 + B10)) ~= 2^t, C tuned for mean relative error.
B10 = 15.0 * 1024.0 - 0.00725 * 1024.0
DVE_FRAC = 0.34  # target fraction of exp columns on DVE


def _body(tc, qT, kT, v1, out):
    nc = tc.nc
    from contextlib import ExitStack

    with ExitStack() as ctx:
        const = ctx.enter_context(tc.tile_pool(name="const", bufs=1))
        qk_pool = ctx.enter_context(tc.tile_pool(name="qk", bufs=HPC))
        v_pool = ctx.enter_context(tc.tile_pool(name="v", bufs=HPC))
        a_pool = ctx.enter_context(tc.tile_pool(name="a", bufs=4))
        o_pool = ctx.enter_context(tc.tile_pool(name="o", bufs=2))
        ps_pool = ctx.enter_context(tc.tile_pool(name="psS", bufs=4, space="PSUM"))
        po_pool = ctx.enter_context(tc.tile_pool(name="psO", bufs=2, space="PSUM"))

        trimask = const.tile([128, 128], F16, name="trimask")
        make_upper_triangular(nc, trimask[:], val=1.0, diag=True)

        # Load all heads upfront; DMA spreads over the whole kernel.
        qts, kts, vts = [], [], []
        for i in range(HPC):
            qt = qk_pool.tile([65, L], F16, tag=f"qt{i}", name=f"qt{i}")
            kt = qk_pool.tile([65, L], F16, tag=f"kt{i}", name=f"kt{i}")
            vt = v_pool.tile([128, NCH * 65], F16, tag=f"vt{i}", name=f"vt{i}")
            for hf in range(2):
                cs = slice(1024 * hf, 1024 * hf + 1024)
                nc.sync.dma_start(kt[:, cs], kT[i][:, cs])
                nc.sync.dma_start(qt[:, cs], qT[i][:, cs])
                vs = slice(8 * 65 * hf, 8 * 65 * hf + 8 * 65)
                nc.sync.dma_start(vt[:, vs], v1[i][:, vs])
            qts.append(qt)
            kts.append(kt)
            vts.append(vt)

        pend = []  # (i, phase, n, c0, w, a_sb) AV pieces awaiting emission
        fin = []  # (i, phase, oT) phases awaiting merge+store emission
        dve_cols = 0
        tot_cols = 0

        def emit_av(u):
            i, phase, n, c0, w, a_sb = u
            qlo = 1024 * phase
            j = (c0 - qlo) // 512
            nc.tensor.matmul(
                _oT[(i, phase)][:, c0 - qlo : c0 - qlo + w],
                lhsT=vts[i][:, n * 65 : n * 65 + 65],
                rhs=a_sb[:, c0 - qlo : c0 - qlo + w],
                start=(n == 0),
                stop=(n == 8 * phase + 4 * j + 3),
            )

        def merge_one():
            ii, pp, oT0 = fin.pop(0)
            while any(u[0] == ii and u[1] == pp for u in pend):
                emit_av(pend.pop(0))
            o_sb = o_pool.tile([65, 1024], F32, tag="osb", name=f"osb{ii}_{pp}")
            nc.vector.tensor_copy(o_sb[:], oT0[:])
            nc.sync.dma_start(out[ii][:, 1024 * pp : 1024 * pp + 1024], o_sb[:])

        _oT = {}
        for i in range(HPC):
            for phase in range(2):
                qlo = 1024 * phase
                qhi = qlo + 1024
                oT = po_pool.tile([65, 1024], F32, tag="oT", name=f"oT{i}_{phase}")
                _oT[(i, phase)] = oT

                for n in range(qhi // 128):
                    q0 = max(128 * n, qlo)
                    if q0 < qlo + 512:
                        pieces = [(q0, qlo + 512 - q0), (qlo + 512, 512)]
                    else:
                        pieces = [(q0, qhi - q0)]
                    a_sb = a_pool.tile(
                        [128, 1024], F16, tag="a", name=f"a{i}_{phase}_{n}"
                    )
                    for pi, (c0, w) in enumerate(pieces):
                        ps = ps_pool.tile(
                            [128, 512], F32, tag="ps", name=f"ps{i}_{phase}_{n}_{pi}"
                        )
                        nc.tensor.matmul(
                            ps[:, 0:w],
                            lhsT=kts[i][:, 128 * n : 128 * n + 128],
                            rhs=qts[i][:, c0 : c0 + w],
                            start=True,
                            stop=True,
                        )
                        asl = slice(c0 - qlo, c0 - qlo + w)
                        use_dve = dve_cols < DVE_FRAC * tot_cols
                        tot_cols += w
                        if use_dve:
                            dve_cols += w
                            nc.vector.tensor_scalar(
                                a_sb[:, asl].bitcast(I16),
                                ps[:, 0:w],
                                1024.0,
                                B10,
                                MULT,
                                ADD,
                            )
                        else:
                            nc.scalar.activation(
                                a_sb[:, asl], ps[:, 0:w], EXP, scale=LN2
                            )
                        if pi == 0 and 128 * n >= qlo:
                            nc.gpsimd.tensor_mul(
                                a_sb[:, q0 - qlo : q0 - qlo + 128],
                                a_sb[:, q0 - qlo : q0 - qlo + 128],
                                trimask[:],
                            )
                        pend.append((i, phase, n, c0, w, a_sb))
                        if len(pend) > 4:
                            emit_av(pend.pop(0))
                fin.append((i, phase, oT))
                # Keep at most one finished phase pending so oT buffers
                # (bufs=2) recycle: emit merge once the NEXT phase is rolling.
                if len(fin) > 1:
                    merge_one()
        for u in pend:
            emit_av(u)
        pend.clear()
        while fin:
            merge_one()


_CACHED = None


def _build():
    global _CACHED
    if _CACHED is not None:
        return _CACHED
    nc = bacc.Bacc("TRN2", target_bir_lowering=False, debug=False)
    qT = nc.dram_tensor("qT", [HPC, 65, L], F16, kind="ExternalInput").ap()
    kT = nc.dram_tensor("kT", [HPC, 65, L], F16, kind="ExternalInput").ap()
    v1 = nc.dram_tensor("v1", [HPC, 128, NCH * 65], F16, kind="ExternalInput").ap()
    out = nc.dram_tensor("out", [HPC, 65, L], F32, kind="ExternalOutput").ap()
    with tile.TileContext(nc) as tc:
        _body(tc, qT, kT, v1, out)
    nc.compile()
    _CACHED = nc
    return nc


def _prep_in_maps(queries, keys, values, tau, delta):
    """Shard + relayout the full inputs into 8 per-core input dicts."""
    queries = np.asarray(queries, dtype=np.float32)
    keys = np.asarray(keys, dtype=np.float32)
    values = np.asarray(values, dtype=np.float32)
    tau = np.asarray(tau, dtype=np.float32)
    delta = np.asarray(delta, dtype=np.float32)

    in_maps = []
    for core in range(NCORES):
        qTs = np.zeros((HPC, 65, L), np.float16)
        kTs = np.zeros((HPC, 65, L), np.float16)
        v1s = np.empty((HPC, 128, NCH * 65), np.float16)
        for slot in range(HPC):
            g = core * HPC + slot
            b, h = divmod(g, H)
            qTs[slot, 0:64] = queries[b, :, h, :].T * (tau[b, 0] * FOLD)
            qTs[slot, 64, :] = 1.0
            kTs[slot, 0:64] = keys[b, :, h, :].T
            kTs[slot, 64, :] = delta[b, :] * FOLD
            v = values[b, :, h, :].reshape(NCH, 128, E).transpose(1, 0, 2)
            vv = np.concatenate([v, np.ones((128, NCH, 1), np.float32)], axis=2)
            v1s[slot] = vv.reshape(128, NCH * 65).astype(np.float16)
        in_maps.append({"qT": qTs, "kT": kTs, "v1": v1s})
    return in_maps


def _assemble(results):
    O = np.empty((B, L, H, E), np.float32)
    for core in range(NCORES):
        o = results[core]["out"]  # [HPC, 65, L]
        for slot in range(HPC):
            g = core * HPC + slot
            b, h = divmod(g, H)
            O[b, :, h, :] = (o[slot, 0:64, :] / o[slot, 64:65, :]).T
    return O


def run(inputs, trace=False, **kwargs):
    from concourse import bass_utils

    nc = _build()
    in_maps = _prep_in_maps(**inputs)
    res = bass_utils.run_bass_kernel_spmd(
        nc, in_maps, core_ids=list(range(NCORES)), trace=trace, **kwargs
    )
    return _assemble(res.results), res


def kernel(**inputs):
    return run(inputs, trace=False)[0]
